# revision 63
# baseline (speedup 1.0000x reference)
"""Trainium2 Bass kernel for nn_CausalDit (sparse frame-causal DiT).

Sharding over 8 NeuronCores (SPMD, one program):
- Token space: 11 frames (6 noisy "zr" + 5 clean "xa"), 257 tokens each
  (256 patch tokens + 1 register/action token). Core c owns rows
  [32c, 32c+32) of every frame plus a replicated copy of each frame's
  leftover token. Per-core token order: [f*32+j for f, j] (352 own rows,
  frame-major), then 11 leftovers, then 1 pad = 364 rows.
- Dense compute (LN/AdaLN-mod, QKV, Wo, GEGLU FFN, gates) is token-sharded;
  attention is head-sharded (8 heads <-> 8 cores), block-sparse at frame
  granularity. An AllToAll ships Q^T/K^T/V^T (d-major, contiguous rows) to
  head owners; attention runs per frame-pair with softmax-without-max
  (denominator via a ones-column in V_aug); a second AllToAll returns
  normalized per-head outputs to token owners.
- All DRAM staging DMAs move contiguous 1456B runs (no per-element
  descriptor scatter); layout fix-ups happen on-chip via strided-AP matmul
  operands, PE transposes (V^T -> token-major V), and a single DVE gather
  for Q.
- Per-frame AdaLN scale/shift/gate tables are broadcast to token rows
  on-chip via one-hot matmuls from a tiny [12, 6, D] table (no big
  replicated table DMAs).
- Matmuls run as float32r (full PE rate for free-dim >= 256) with fp32
  PSUM accumulation.
- Host (numpy, fp32) does only tiny prep: patchify + patch matmul,
  embedding gathers, per-frame AdaLN tables, bias folds (b_k dropped -
  softmax-shift-invariant; b_v folded into b_o), and the final unpatch.
"""
import numpy as np

import concourse.bass as bass
import concourse.mybir as mybir
import concourse.tile as tile
from concourse import bacc
from concourse.bass_utils import run_bass_kernel_spmd
from concourse.masks import make_identity

# ---- model constants (hardcoded from the problem spec) ----
P2 = 2; NH = 8; NW = 4; NB = 6; D = 512; HID = 2048
HH = 32; WW = 32; C = 3; DUR = 6
DH = D // NH          # 64
NZ = DUR              # 6 zr frames
NX = DUR - 1          # 5 xa frames
NF = NZ + NX          # 11 frames
S = (HH // P2) * (WW // P2)   # 256
TPF = S + 1           # 257 tokens/frame
NCORE = 8
OWN = 32              # owned rows per frame per core
NOWN = NF * OWN       # 352 own rows
TOK = NOWN + NF       # 363 real rows per core
TOKP = TOK + 1        # padded to 364
TPQ = 258             # q columns per frame: 256 own + leftover + pad
NFP = NF + 1          # leftover rows padded to 12
VA = DH + 2           # V_aug cols: 64 v + 1 ones + 1 pad = 66
CORE_IDS = list(range(NCORE))
TOKT = [(0, 128), (128, 128), (256, 108)]   # token tiles of 364
F32 = mybir.dt.float32
F32R = mybir.dt.float32r
BF16 = mybir.dt.bfloat16
AX = mybir.AxisListType.X
ALU = mybir.AluOpType
ACTF = mybir.ActivationFunctionType

F16 = mybir.dt.float16
F8 = mybir.dt.float8e3                   # TRN E3M4: 4 mantissa bits, +-15.5
DTA = BF16                               # attention transport dtype
DTF = F16                                # FFN hp / W_ffout dtype
DT = DTA
DTT = DTA                                # PE-transpose dtype (V path)
# GEGLU a/g matmuls run double-pumped fp8 (e3m4): weights are host-scaled
# by WSC (so sigma~0.02 weights land in e3m4 normals), activations cast
# unscaled, and the 1/WSC folds into the gelu scale / a-bias / W_ffout
import os as _os_mod
# fp8 GEGLU measured rel-err ~3e-2/block: the architecture has no residual
# skip (x is replaced by gate*FFN each block), so activation quantization
# compounds. Keep f16 unless explicitly enabled.
FP8_AG = bool(_os_mod.environ.get("KERNEL_FP8"))
WSC = 16.0 if FP8_AG else 1.0
# xn2 is produced pre-divided by XSC (folded into the mod2 tables) so the
# e3m4 cast cannot saturate (|xn2| can reach ~12 vs e3m4 max 15.5)
XSC = 4.0 if FP8_AG else 1.0
ASC = WSC / XSC
WG_DT = F8 if FP8_AG else F16
XN2_DT = F8 if FP8_AG else F16


def _kv_frames(fq):
    """Global kv-frame indices for q-frame fq (zr: 0..5, xa: 6..10)."""
    if fq < NZ:
        return [fq] + [NZ + j for j in range(max(0, fq - NW), min(fq, NX))]
    return list(range(NZ, fq + 1))


def _R(ap):
    return ap.bitcast(F32R)


def _M(ap):
    """Matmul-operand view: f32 storage is bitcast to f32r; bf16 passes through."""
    return ap.bitcast(F32R) if ap.dtype == F32 else ap


def _ps32(ap):
    """Copy-source view of a PE-transpose PSUM tile (f32r reads as f32)."""
    return ap.bitcast(F32) if ap.dtype == F32R else ap


_CACHE = {}
LAST_RESULT = None


def _build(n_blocks, repeat=1):
    import os as _os
    skip_coll = bool(_os.environ.get("KERNEL_SKIP_COLL"))
    nc = bacc.Bacc("TRN2", target_bir_lowering=False, debug=False,
                   num_devices=NCORE)
    x0_e = nc.declare_dram_parameter("x0", [TOKP, D], F32, isOutput=False)
    lb_e = nc.declare_dram_parameter("lbias", [NFP, NFP], F32, isOutput=False)
    xout_e = nc.declare_dram_parameter("xout", [TOKP, D], F32, isOutput=True)
    ext = []
    for i in range(n_blocks):
        e = dict(
            wqkvo=nc.declare_dram_parameter(f"wqkvo{i}", [128, 16, D], DTA, isOutput=False),
            wg=nc.declare_dram_parameter(f"wg{i}", [128, 4, 2 * HID], WG_DT, isOutput=False),
            wf=nc.declare_dram_parameter(f"wf{i}", [128, 16, D], DTF, isOutput=False),
            bq=nc.declare_dram_parameter(f"bq{i}", [64, 1], F32, isOutput=False),
            bgl=nc.declare_dram_parameter(f"bgl{i}", [128, 32], F32, isOutput=False),
            bop=nc.declare_dram_parameter(f"bop{i}", [1, D], F16, isOutput=False),
            bff=nc.declare_dram_parameter(f"bff{i}", [1, D], F16, isOutput=False),
            # token-broadcast AdaLN tables, host-precomputed: [p, tt*6+v, d]
            tabb=nc.declare_dram_parameter(f"tabb{i}", [128, 18, D], F16, isOutput=False),
        )
        ext.append(e)

    with tile.TileContext(nc) as tc:
        with (
            tc.tile_pool(name="const", bufs=1) as cpool,
            tc.tile_pool(name="blk", bufs=2) as bpool,
            tc.tile_pool(name="xp", bufs=2) as xpool,
            tc.tile_pool(name="wp", bufs=2) as wpool,
            tc.tile_pool(name="wgs", bufs=int(_os.environ.get("KERNEL_WGS", 2))) as wgpool,
            tc.tile_pool(name="act", bufs=1) as apool,
            tc.tile_pool(name="at", bufs=1) as atpool,
            tc.tile_pool(name="hp", bufs=int(_os.environ.get("KERNEL_HP", 2))) as hpool,
            tc.tile_pool(name="sc", bufs=int(_os.environ.get("KERNEL_SC", 2))) as scpool,
            tc.tile_pool(name="pt", bufs=int(_os.environ.get("KERNEL_PT", 6))) as ptpool,
            tc.tile_pool(name="psA", bufs=int(_os.environ.get("KERNEL_PSA", 2)), space="PSUM") as pspool,
            tc.tile_pool(name="psB", bufs=int(_os.environ.get("KERNEL_PSB", 4)), space="PSUM") as big3,
            tc.tile_pool(name="psT", bufs=2, space="PSUM") as pstab,
            tc.tile_pool(name="dram", bufs=1, space="DRAM") as dpool,
        ):
            # warmup collective: absorbs the ~25us first-call ncfw cost and
            # the cross-core entry barrier while the PE does block-0 prep
            warm_sb = cpool.tile([NCORE, 16], DT)
            nc.vector.memset(warm_sb[:NCORE], 0.0)
            warm_s = dpool.tile([NCORE, 16], DT, tag="warms")
            warm_r = dpool.tile([NCORE, 16], DT, tag="warmr")
            nc.sync.dma_start(warm_s[:], warm_sb[:NCORE])
            if not skip_coll:
                nc.gpsimd.collective_compute(
                    "AllToAll", ALU.bypass, replica_groups=[CORE_IDS],
                    ins=[warm_s.opt()], outs=[warm_r.opt()])

            ident32 = cpool.tile([128, 128], F32)
            make_identity(nc, ident32[:])
            ident = cpool.tile([128, 128], F32R)
            nc.vector.tensor_copy(ident[:], ident32[:])
            identT = cpool.tile([128, 128], DTT)
            nc.vector.tensor_copy(identT[:], ident32[:])
            eps = cpool.tile([128, 1], F32)
            nc.vector.memset(eps[:], 1e-5)
            onescol = cpool.tile([1, 128], F16)
            nc.vector.memset(onescol[0:1], 1.0)
            lbias = cpool.tile([NFP, NFP], F32)
            nc.sync.dma_start(lbias[:NFP], lb_e[:])

            for rr in range(repeat):
              x = xpool.tile([128, 3, D], F32, tag="x", name=f"x_{rr}")
              nc.sync.dma_start(x[:, 0:2, :], x0_e[0:256, :].rearrange("(t r) d -> r t d", r=128))
              nc.sync.dma_start(x[:108, 2, :], x0_e[256:364, :])

              for i in range(n_blocks):
                  e = ext[i]
                  wqkvo = wpool.tile([128, 16, D], DTA, tag="wqkvo")
                  nc.sync.dma_start(wqkvo[:], e["wqkvo"][:])
                  bq_sb = bpool.tile([64, 1], F32, tag="bq")
                  nc.sync.dma_start(bq_sb[:64], e["bq"][:])
                  bgl_sb = bpool.tile([128, 32], F32, tag="bgl")
                  nc.sync.dma_start(bgl_sb[:], e["bgl"][:])
                  bop_sb = bpool.tile([1, D], F16, tag="bop")
                  nc.sync.dma_start(bop_sb[0:1], e["bop"][:])
                  bff_sb = bpool.tile([1, D], F16, tag="bff")
                  nc.sync.dma_start(bff_sb[0:1], e["bff"][:])
                  tabb = bpool.tile([128, 18, D], F16, tag="tabb")
                  nc.sync.dma_start(tabb[:], e["tabb"][:])

                  # last block: xa tokens and register/leftover rows are dead
                  # past attention -> keep only zr own rows (cols 0:192)
                  last = i == n_blocks - 1 and rr == repeat - 1
                  tts_live = ([(0, (0, 128)), (1, (128, 64))] if last
                              else list(enumerate(TOKT)))
                  # split-stream row passes: A = xa own + all leftover rows
                  # (token cols 192:364), B = zr own rows (cols 0:192)
                  MOD_A = [(1, 64, 128), (2, 0, 108)]
                  MOD_B = [(0, 0, 128), (1, 0, 64)]
                  WO_A = [] if last else [(1, 64, 128), (2, 0, 96)]
                  WO_B = MOD_B + ([] if last else [(2, 96, 108)])

                  def ln_mod(srcs, v_s, v_t, xn_f32, rows):
                      """xn = LN(srcs[tt])*tab[v_s] + tab[v_t] (fp32).

                      rsqrt via scalar Ln/Exp (DVE reciprocal is ~1.1us);
                      the normalize itself runs on the scalar engine with
                      per-partition scale/bias APs, leaving DVE only the
                      two table ops."""
                      for tt, rl, rh in rows:
                          s_tb = tabb[rl:rh, tt * 6 + v_s, :]
                          t_tb = tabb[rl:rh, tt * 6 + v_t, :]
                          xt = srcs[tt][rl:rh, :]
                          st6 = scpool.tile([128, 6], F32, tag="st6")
                          nc.vector.bn_stats(st6[rl:rh], xt)
                          mv = scpool.tile([128, 2], F32, tag="mv")
                          nc.vector.bn_aggr(mv[rl:rh], st6[rl:rh])
                          lnv = scpool.tile([128, 1], F32, tag="lnv")
                          nc.scalar.activation(lnv[rl:rh], mv[rl:rh, 1:2], ACTF.Ln,
                                               bias=eps[rl:rh])
                          rin = scpool.tile([128, 1], F32, tag="rin")
                          nc.scalar.activation(rin[rl:rh], lnv[rl:rh], ACTF.Exp,
                                               scale=-0.5)
                          # xc overlaps the scalar Ln/Exp pair
                          xc = scpool.tile([128, D], F32, tag="xc")
                          nc.vector.tensor_scalar(xc[rl:rh], xt, mv[rl:rh, 0:1], None,
                                                  op0=ALU.subtract)
                          tmp = scpool.tile([128, D], F32, tag="lntmp")
                          nc.vector.scalar_tensor_tensor(
                              tmp[rl:rh], s_tb, rin[rl:rh], xc[rl:rh],
                              op0=ALU.mult, op1=ALU.mult)
                          nc.vector.tensor_add(xn_f32[rl:rh, tt, :], tmp[rl:rh],
                                               t_tb)

                  def transpose_tok(xn_f32, xnT, rows):
                      """xn [128,3,D] f32 -> xnT [128,4,TOKP] (d-major)."""
                      for tt, rl, rh in rows:
                          r0 = TOKT[tt][0]
                          n = rh - rl
                          pst = big3.tile([128, 512], F32R, tag="ps3")
                          for kd in range(4):
                              nc.tensor.transpose(
                                  pst[:, 128 * kd + rl:128 * kd + rh],
                                  xn_f32[rl:rh, tt, kd * 128:(kd + 1) * 128],
                                  ident[rl:rh, rl:rh])
                          nc.vector.tensor_copy(
                              xnT[:, :, r0 + rl:r0 + rh],
                              pst[:].rearrange("p (k c) -> p k c", k=4)[:, :, rl:rh].bitcast(F32))

                  def qkv_pass(c0, c1, a2a_s_h, qkvt_h):
                      w = c1 - c0
                      for m in range(4):
                          for t in range(3):
                              ps = pspool.tile([128, TOKP], F32, tag="psbig")
                              for k in range(4):
                                  nc.tensor.matmul(ps[:, 0:w],
                                                   wqkvo[:, 4 * t + k, m * 128:(m + 1) * 128],
                                                   xnT[:, k, c0:c1], start=(k == 0), stop=(k == 3))
                              nc.vector.tensor_copy(qkvt_h[:, t, m, :], ps[:, 0:w])
                          nc.sync.dma_start(
                              a2a_s_h[2 * m:2 * m + 2, :, :, :].rearrange("d r t x -> (d r) t x"),
                              qkvt_h[:, :, m, :])

                  def g1x_pass(rows):
                      for tt, rl, rh in rows:
                          nc.vector.tensor_mul(g1x_t[tt][rl:rh],
                                               xn1[rl:rh, tt, :].bitcast(F32),
                                               tabb[rl:rh, tt * 6 + 2, :])

                  XAC = TOKP - 192      # 172 cols: xa own + leftovers + pad
                  ZRC = 192             # zr own cols

                  # ---- mod1 pass A (xa + leftovers) -> QKV-A -> A2A#1 ----
                  xn1 = apool.tile([128, 3, D], F32R, tag="xn1")
                  xnT = apool.tile([128, 4, TOKP], DT, tag="xnT")
                  xsrc = [x[:, 0, :], x[:, 1, :], x[:, 2, :]]
                  ln_mod(xsrc, 0, 1, xn1, MOD_A)
                  transpose_tok(xn1, xnT, MOD_A)
                  a2a_sA = dpool.tile([NCORE, 64, 3, XAC], DT, tag=f"a2asA{rr}_{i}")
                  a2a_rA = dpool.tile([NCORE, 64, 3, XAC], DT, tag=f"a2arA{rr}_{i}")
                  qkvtA = apool.tile([128, 3, 4, XAC], DT, tag="qkvtA")
                  qkv_pass(192, TOKP, a2a_sA, qkvtA)
                  if skip_coll:
                      nc.sync.dma_start(a2a_rA[:], a2a_sA[:])
                  else:
                      nc.gpsimd.collective_compute(
                          "AllToAll", ALU.bypass, replica_groups=[CORE_IDS],
                          ins=[a2a_sA.opt()], outs=[a2a_rA.opt()])

                  # ---- overlap A2A#1: gate1-A, mod1 pass B, QKV-B, A2A#2 ----
                  g1x_t = {tt: apool.tile([128, D], F32, tag=f"g1x_{tt}",
                                          name=f"g1x_{tt}")
                           for tt in (range(2) if last else range(3))}
                  if not last:
                      g1x_pass(MOD_A)
                  ln_mod(xsrc, 0, 1, xn1, MOD_B)
                  transpose_tok(xn1, xnT, MOD_B)
                  a2a_sB = dpool.tile([NCORE, 64, 3, ZRC], DT, tag=f"a2asB{rr}_{i}")
                  a2a_rB = dpool.tile([NCORE, 64, 3, ZRC], DT, tag=f"a2arB{rr}_{i}")
                  qkvtB = apool.tile([128, 3, 4, ZRC], DT, tag="qkvtB")
                  qkv_pass(0, 192, a2a_sB, qkvtB)
                  if skip_coll:
                      nc.sync.dma_start(a2a_rB[:], a2a_sB[:])
                  else:
                      nc.gpsimd.collective_compute(
                          "AllToAll", ALU.bypass, replica_groups=[CORE_IDS],
                          ins=[a2a_sB.opt()], outs=[a2a_rB.opt()])
                  g1x_pass(MOD_B)

                  # ---- receive A then B (V first: PE transposes start early) --
                  qkvsA = atpool.tile([64, 3, NCORE, XAC], DT, tag="qkvsA")
                  for t in (2, 1, 0):
                      nc.sync.dma_start(qkvsA[:, t],
                                        a2a_rA[:, :, t, :].rearrange("s r x -> r s x"))
                  qkvsB = atpool.tile([64, 3, NCORE, ZRC], DT, tag="qkvsB")
                  for t in (2, 1, 0):
                      nc.sync.dma_start(qkvsB[:, t],
                                        a2a_rB[:, :, t, :].rearrange("s r x -> r s x"))
                  qsA, ksA, vsA = qkvsA[:, 0], qkvsA[:, 1], qkvsA[:, 2]
                  qsB, ksB, vsB = qkvsB[:, 0], qkvsB[:, 1], qkvsB[:, 2]

                  # prefetch the first GEGLU weight piece now: emitted later,
                  # its DMA trigger would queue behind collective-dependent
                  # loads in the sync FIFO
                  def load_piece(p):
                      wgp = wgpool.tile([128, 4, 512], WG_DT, tag="wgp")
                      nc.sync.dma_start(wgp[:], e["wg"][:, :, 512 * p:512 * (p + 1)])
                      wfp = wgpool.tile([128, 2, D], DTF, tag="wfp")
                      nc.sync.dma_start(wfp[:], e["wf"][:, 2 * p:2 * (p + 1), :])
                      return wgp, wfp
                  piece = {0: load_piece(0)}

                  # ---- assemble A half: xa frames + leftovers ----
                  vtA = atpool.tile([64, NF, S], DT, tag="vtA")
                  ktA = atpool.tile([64, NF, S], DT, tag="ktA")
                  qtA = atpool.tile([64, NF, TPQ], DT, tag="qtA")
                  v_a = atpool.tile([128, 2 * NF, VA], DT, tag="va")
                  nc.vector.memset(v_a[:, :, DH], 1.0)
                  nc.vector.memset(v_a[:, :, DH + 1], 0.0)
                  nc.vector.tensor_copy(
                      vtA[:, NZ:NF].rearrange("p f (s j) -> p f s j", s=NCORE),
                      vsA[:, :, 0:NX * 32].rearrange("p s (f j) -> p f s j", f=NX))
                  for f in range(NZ, NF):
                      pst = big3.tile([128, 128], DTT, tag="ps3")
                      for t2 in range(2):
                          nc.tensor.transpose(
                              pst[:, 64 * t2:64 * t2 + 64],
                              vtA[:, f, 128 * t2:128 * (t2 + 1)],
                              identT[:64, :64])
                      nc.vector.tensor_copy(
                          v_a[:, 2 * f:2 * f + 2, 0:DH],
                          _ps32(pst[:].rearrange("p (t c) -> p t c", t=2)))
                  v_l = atpool.tile([NFP, VA], DT, tag="vl")
                  nc.vector.memset(v_l[:NFP, DH:DH + 1], 1.0)
                  nc.vector.memset(v_l[:NFP, DH + 1:DH + 2], 0.0)
                  pstl = big3.tile([NFP, 64], DTT, tag="ps3")
                  nc.tensor.transpose(pstl[:NFP, :64], _M(vsA[:, 7, NX * 32:XAC]),
                                      identT[:64, :64])
                  nc.vector.tensor_copy(v_l[:NFP, 0:DH], _ps32(pstl[:NFP, :64]))
                  nc.vector.tensor_copy(
                      ktA[:, NZ:NF].rearrange("p f (s j) -> p f s j", s=NCORE),
                      ksA[:, :, 0:NX * 32].rearrange("p s (f j) -> p f s j", f=NX))
                  ktl = atpool.tile([64, NFP], DT, tag="ktl")
                  nc.vector.tensor_copy(ktl[:64, :], ksA[:, 7, NX * 32:XAC])
                  nc.vector.tensor_scalar(
                      qtA[:, NZ:NF, 0:S].rearrange("p f (s j) -> p f s j", s=NCORE),
                      qsA[:, :, 0:NX * 32].rearrange("p s (f j) -> p f s j", f=NX),
                      bq_sb[:64, 0:1], None, op0=ALU.add)
                  nc.vector.tensor_scalar(qtA[:, :, S], qsA[:, 7, NX * 32:NX * 32 + NF],
                                          bq_sb[:64, 0:1], None, op0=ALU.add)
                  nc.vector.memset(qtA[:, :, S + 1], 0.0)

                  # ---- attention ----
                  otA = atpool.tile([64, NF, TPQ], DT, tag="otA")

                  def attn_frame(fq):
                      kvf = _kv_frames(fq)
                      n_main = 2 * len(kvf)
                      ps_o = pspool.tile([VA, TPQ], F32, tag="psbig")
                      ps_l = big3.tile([NFP, TPQ], F32, tag="ps3")
                      nc.tensor.matmul(ps_l[:NFP], _M(ktl[:64, :]),
                                       _M(qtA[:, fq, :]), start=True, stop=True)
                      pl = ptpool.tile([NFP, TPQ], DTT, tag="pt")
                      nc.scalar.activation(pl[:NFP, :], ps_l[:NFP, :], ACTF.Exp,
                                           scale=0.125, bias=lbias[:NFP, fq:fq + 1])
                      nc.tensor.matmul(ps_o[:], _M(v_l[:NFP, :]), pl[:NFP],
                                       start=True, stop=False)
                      av_i = 0
                      for fi in kvf:
                          for t2 in range(2):
                              ps_s = big3.tile([128, TPQ], F32, tag="ps3")
                              nc.tensor.matmul(
                                  ps_s[:],
                                  ktA[:, fi, 128 * t2:128 * (t2 + 1)],
                                  _M(qtA[:, fq, :]), start=True, stop=True)
                              pt = ptpool.tile([128, TPQ], DTT, tag="pt")
                              nc.scalar.activation(pt[:], ps_s[:], ACTF.Exp, scale=0.125)
                              nc.tensor.matmul(ps_o[:], _M(v_a[:, 2 * fi + t2, :]), pt[:],
                                               start=False, stop=(av_i == n_main - 1))
                              av_i += 1
                      # 1/denominator; broadcast on gpsimd (idle between
                      # collective waits at this point)
                      rc = scpool.tile([1, TPQ], F32, tag="rc")
                      nc.vector.reciprocal(rc[0:1], ps_o[DH:DH + 1, :])
                      bc = scpool.tile([64, TPQ], F32, tag="bc")
                      nc.gpsimd.partition_broadcast(bc[:64], rc[0:1, :])
                      nc.vector.tensor_mul(otA[:, fq, :], ps_o[0:DH, :], bc[:64])

                  # xa self-attention first: runs entirely out of half A,
                  # covering A2A#2's flight
                  if not last:
                      for fq in range(NZ, NF):
                          attn_frame(fq)
                      # ---- bk#1: ship xa outputs while zr attention runs ----
                      bk_sendA = atpool.tile([64, NCORE, NX * 32], DT, tag="bksA")
                      nc.vector.tensor_copy(
                          bk_sendA[:].rearrange("p d (f j) -> p d f j", f=NX),
                          otA[:, NZ:NF, 0:S].rearrange("p f (d j) -> p d f j", d=NCORE))
                      bk_sA = dpool.tile([NCORE, 64, NX * 32], DT, tag=f"bksA{rr}_{i}")
                      bk_rA = dpool.tile([NCORE, 64, NX * 32], DT, tag=f"bkrA{rr}_{i}")
                      nc.sync.dma_start(bk_sA.rearrange("d r x -> r d x"), bk_sendA[:])
                      if skip_coll:
                          nc.sync.dma_start(bk_rA[:], bk_sA[:])
                      else:
                          nc.gpsimd.collective_compute(
                              "AllToAll", ALU.bypass, replica_groups=[CORE_IDS],
                              ins=[bk_sA.opt()], outs=[bk_rA.opt()])
                      # load xa attention outputs NOW: emitted later, this DMA
                      # trigger would queue behind bk#2's staging in the sync
                      # FIFO and stall Wo pass A past bk#2
                      xaT = apool.tile([128, 4, TOKP], DT, tag="qt")
                      nc.sync.dma_start(
                          xaT[:, :, 192:352],
                          bk_rA.rearrange("(c p2) r x -> (p2 r) c x", c=4))

                  # ---- assemble B half: zr frames ----
                  nc.vector.tensor_copy(
                      vtA[:, 0:NZ].rearrange("p f (s j) -> p f s j", s=NCORE),
                      vsB[:, :, 0:NZ * 32].rearrange("p s (f j) -> p f s j", f=NZ))
                  for f in range(NZ):
                      pst = big3.tile([128, 128], DTT, tag="ps3")
                      for t2 in range(2):
                          nc.tensor.transpose(
                              pst[:, 64 * t2:64 * t2 + 64],
                              vtA[:, f, 128 * t2:128 * (t2 + 1)],
                              identT[:64, :64])
                      nc.vector.tensor_copy(
                          v_a[:, 2 * f:2 * f + 2, 0:DH],
                          _ps32(pst[:].rearrange("p (t c) -> p t c", t=2)))
                  nc.vector.tensor_copy(
                      ktA[:, 0:NZ].rearrange("p f (s j) -> p f s j", s=NCORE),
                      ksB[:, :, 0:NZ * 32].rearrange("p s (f j) -> p f s j", f=NZ))
                  nc.vector.tensor_scalar(
                      qtA[:, 0:NZ, 0:S].rearrange("p f (s j) -> p f s j", s=NCORE),
                      qsB[:, :, 0:NZ * 32].rearrange("p s (f j) -> p f s j", f=NZ),
                      bq_sb[:64, 0:1], None, op0=ALU.add)

                  # zr attention (covers bk#1's flight)
                  for fq in range(NZ):
                      attn_frame(fq)

                  # ---- bk#2: zr outputs + all leftover outputs ----
                  BKC = ZRC if last else ZRC + NFP
                  bk_sendB = atpool.tile([64, NCORE, ZRC + NFP], DT, tag="bksB")
                  nc.vector.tensor_copy(
                      bk_sendB[:, :, 0:ZRC].rearrange("p d (f j) -> p d f j", f=NZ),
                      otA[:, 0:NZ, 0:S].rearrange("p f (d j) -> p d f j", d=NCORE))
                  if not last:
                      nc.vector.tensor_copy(
                          bk_sendB[:, :, ZRC:ZRC + NF],
                          otA[:, None, :, S].broadcast_to([64, NCORE, NF]))
                      nc.vector.memset(bk_sendB[:, :, ZRC + NF], 0.0)
                  bk_sB = dpool.tile([NCORE, 64, BKC], DT, tag=f"bksB{rr}_{i}")
                  bk_rB = dpool.tile([NCORE, 64, BKC], DT, tag=f"bkrB{rr}_{i}")
                  nc.sync.dma_start(bk_sB.rearrange("d r x -> r d x"),
                                    bk_sendB[:, :, 0:BKC])
                  if skip_coll:
                      nc.sync.dma_start(bk_rB[:], bk_sB[:])
                  else:
                      nc.gpsimd.collective_compute(
                          "AllToAll", ALU.bypass, replica_groups=[CORE_IDS],
                          ins=[bk_sB.opt()], outs=[bk_rB.opt()])

                  # ---- Wo/mod2 pass A during bk#2's flight, pass B after ----
                  if last:
                      xaT = apool.tile([128, 4, TOKP], DT, tag="qt")
                  nc.sync.dma_start(
                      xaT[:, :, 0:192],
                      bk_rB[:, :, 0:ZRC].rearrange("(c p2) r x -> (p2 r) c x", c=4))
                  if not last:
                      nc.sync.dma_start(
                          xaT[:, :, 352:364],
                          bk_rB[:, :, ZRC:ZRC + NFP].rearrange("(c p2) r x -> (p2 r) c x", c=4))

                  x2g = {tt: apool.tile([128, D], F32, tag=f"x2_{tt}",
                                        name=f"x2_{tt}")
                         for tt in (range(2) if last else range(3))}
                  xn2 = apool.tile([128, 3, D], F32R, tag="xn1")
                  xn2T = apool.tile([128, 4, TOKP], XN2_DT, tag="xn2T")

                  def wo_pass(rows):
                      for tt, rl, rh in rows:
                          r0 = TOKT[tt][0]
                          n = rh - rl
                          # PSUM matmul out base must be 0/32/64; DVE bridges
                          # the offset for the leftover sliver (rows 96:108)
                          ob = rl if rl in (0, 32, 64) else 64
                          pso = pspool.tile([128, D], F32, tag="psbig")
                          # bo' seeds the accumulator via a ones-row matmul
                          nc.tensor.matmul(pso[ob:ob + n], onescol[0:1, 0:n],
                                           bop_sb[0:1, :], start=True, stop=False)
                          for k in range(4):
                              nc.tensor.matmul(pso[ob:ob + n],
                                               _M(xaT[:, k, r0 + rl:r0 + rh]),
                                               wqkvo[:, 12 + k, :], start=False, stop=(k == 3))
                          nc.vector.tensor_add(x2g[tt][rl:rh, :], pso[ob:ob + n],
                                               g1x_t[tt][rl:rh, :])

                  wo_pass(WO_A)
                  ln_mod(x2g, 3, 4, xn2, WO_A)
                  transpose_tok(xn2, xn2T, WO_A)
                  wo_pass(WO_B)
                  ln_mod(x2g, 3, 4, xn2, WO_B)
                  # PE base-partition must be 0/32/64: widen the leftover
                  # sliver's transpose to start at 64 (rows 64:96 rewritten
                  # with identical values)
                  TR_B = MOD_B + ([] if last else [(2, 64, 108)])
                  transpose_tok(xn2, xn2T, TR_B)

                  # ---- GEGLU + FF out, streamed in 8 weight pieces ----
                  psf = {tt: big3.tile([128, D], F32, tag="ps3", name=f"psf{rr}_{i}_{tt}")
                         for tt, _ in tts_live}
                  # b_ffout seeds each accumulator via a ones-row matmul
                  for tt, (r0, p_) in tts_live:
                      nc.tensor.matmul(psf[tt][:p_], onescol[0:1, 0:p_],
                                       bff_sb[0:1, :], start=True, stop=False)
                  tok_rs = ((0, 192),) if last else ((0, TOKP),)
                  # psf row-ranges with matching hp column spans
                  PSF_A = [(1, 64, 128), (2, 0, 108)]
                  PSF_B = [(0, 0, 128), (1, 0, 64)]
                  PSF_FULL = [(tt, 0, p_) for tt, (r0, p_) in
                              sorted(tts_live, key=lambda z: -z[0])]
                  def geglu_stage(mm, j, wgp, wfp, psa, psg, gel, hp, cols, rows):
                      for c0, c1 in cols:
                          for k in range(4):
                              nc.tensor.matmul(psa[:, c0:c1], wgp[:, k, 256 * j:256 * j + 128],
                                               xn2T[:, k, c0:c1], start=(k == 0), stop=(k == 3))
                          for k in range(4):
                              nc.tensor.matmul(psg[:, c0:c1], wgp[:, k, 256 * j + 128:256 * j + 256],
                                               xn2T[:, k, c0:c1], start=(k == 0), stop=(k == 3))
                          nc.scalar.activation(gel[:, c0:c1], psg[:, c0:c1], ACTF.Gelu,
                                               scale=1.0 / ASC,
                                               bias=bgl_sb[:, 2 * mm + 1:2 * mm + 2])
                          nc.vector.scalar_tensor_tensor(hp[:, c0:c1], psa[:, c0:c1],
                                                         bgl_sb[:, 2 * mm:2 * mm + 1], gel[:, c0:c1],
                                                         op0=ALU.add, op1=ALU.mult)
                      for tt, rl, rh in rows:
                          r0 = TOKT[tt][0]
                          assert rl in (0, 32, 64)
                          nc.tensor.matmul(psf[tt][rl:rh],
                                           _M(hp[:, r0 + rl:r0 + rh]),
                                           wfp[:, j, :],
                                           start=False, stop=(mm == 15))

                  for p in range(8):
                      wgp, wfp = piece.pop(p)
                      if p + 1 < 8:
                          piece[p + 1] = load_piece(p + 1)
                      for j in range(2):
                          mm = 2 * p + j
                          # pieces 0-1: A columns run during bk#2's flight
                          # (xn2T-A is ready), B columns after bk#2 lands
                          if p < 2 and not last:
                              stages = [(((192, TOKP),), PSF_A),
                                        (((0, 192),), PSF_B)]
                          else:
                              stages = [(tok_rs, PSF_FULL)]
                          psa = pspool.tile([128, TOKP], F32, tag="psbig")
                          # pstab's banks are idle through the GEGLU body;
                          # using them for psg doubles the pipeline depth
                          psg = pstab.tile([128, TOKP], F32, tag="pstab")
                          gel = scpool.tile([128, TOKP], F32, tag="gel")
                          hp = hpool.tile([128, TOKP], DTF, tag="hp")
                          for cols, rows in stages:
                              geglu_stage(mm, j, wgp, wfp, psa, psg, gel, hp,
                                          cols, rows)

                  x_new = xpool.tile([128, 3, D], F32, tag="x")
                  for tt, (r0, p_) in sorted(tts_live, key=lambda z: -z[0]):
                      nc.vector.tensor_mul(x_new[:p_, tt, :], psf[tt][:p_],
                                           tabb[:p_, tt * 6 + 5, :])
                  x = x_new
                  if _os.environ.get("KERNEL_BLOCK_BARRIER"):
                      # optional scheduling barrier between blocks (collective
                      # ordering is data-enforced; barrier-free validated on HW)
                      tc.strict_bb_all_engine_barrier()

            # only zr own rows (token slots 0:192) survive the last block
            nc.sync.dma_start(xout_e[0:128, :], x[:, 0, :])
            nc.sync.dma_start(xout_e[128:192, :], x[:64, 1, :])
    nc.compile()
    return nc


# ----------------------------------------------------------------------
# host side
# ----------------------------------------------------------------------
def _silu(x):
    return x / (1.0 + np.exp(-x))


def _frame_of():
    """frame index of each per-core token slot (12 = pad/zero row)."""
    fr = np.full(TOKP, NF, np.int64)
    fr[:NOWN] = np.arange(NOWN) // OWN
    fr[NOWN:TOK] = np.arange(NF)
    return fr


def _host_prep(inputs, n_blocks):
    f32 = np.float32
    z = np.asarray(inputs['z'], f32)
    frames = np.asarray(inputs['frames'], f32)
    actions = np.asarray(inputs['actions'])
    ts = np.asarray(inputs['ts'])

    def patch(xx):
        b, dur, c, h, w = xx.shape
        xx = xx.reshape(b, dur, c, h // P2, P2, w // P2, P2)
        xx = xx.transpose(0, 1, 3, 5, 2, 4, 6).reshape(b, dur, (h // P2) * (w // P2), c * P2 * P2)
        return xx @ np.asarray(inputs['W_patch'], f32) + np.asarray(inputs['b_patch'], f32)

    pe = np.asarray(inputs['pe_grid'], f32)
    zt = patch(z)[0] + pe[None]
    xt = patch(frames)[0] + pe[None]
    reg = np.asarray(inputs['registers'], f32)
    aemb = np.asarray(inputs['action_emb'], f32)
    temb = np.asarray(inputs['time_emb'], f32)
    a = aemb[actions[0]]

    ft = np.zeros((NF, TPF, D), f32)
    for f in range(NZ):
        ft[f, :S] = zt[f]
        ft[f, S] = reg[0]
    for f in range(NX):
        ft[NZ + f, :S] = xt[f]
        ft[NZ + f, S] = a[f]

    cond = np.zeros((NF, D), f32)
    for f in range(NZ):
        cond[f] = temb[ts[0, f]]
    for f in range(NX):
        cond[NZ + f] = temb[0]
    sc = _silu(cond)

    blocks = []
    for i in range(n_blocks):
        m1 = sc @ np.asarray(inputs['W_mod1'][i], f32) + np.asarray(inputs['b_mod1'][i], f32)
        s1, t1 = np.split(m1, 2, -1)
        m2 = sc @ np.asarray(inputs['W_mod2'][i], f32) + np.asarray(inputs['b_mod2'][i], f32)
        s2, t2 = np.split(m2, 2, -1)
        g1 = cond @ np.asarray(inputs['W_g1'][i], f32) + np.asarray(inputs['b_g1'][i], f32)
        g2 = cond @ np.asarray(inputs['W_g2'][i], f32) + np.asarray(inputs['b_g2'][i], f32)
        bo_p = (np.asarray(inputs['b_o'][i], f32)
                + np.asarray(inputs['b_v'][i], f32) @ np.asarray(inputs['W_o'][i], f32))
        tabf = np.zeros((NFP + 1, 6, D), f32)
        tabf[:NF] = np.stack([1.0 + s1, t1, g1,
                              (1.0 + s2) / XSC, t2 / XSC, g2], 1)
        # token-broadcast form: row p of group tt gets frame fr(tt*128+p)
        fr = _frame_of()
        frp = np.full(3 * 128, NFP, np.int64)
        frp[:TOKP] = np.minimum(fr, NFP)
        tabb = tabf[frp.reshape(3, 128)]            # [3, 128, 6, D]
        tabb = np.ascontiguousarray(
            tabb.transpose(1, 0, 2, 3).reshape(128, 18, D)).astype(np.float16)

        def chunk(w, kparts):
            K, N = w.shape
            return np.ascontiguousarray(
                np.asarray(w, f32).reshape(kparts, 128, N).swapaxes(0, 1))

        wq = chunk(np.asarray(inputs['W_q'][i]), 4)
        wk = chunk(np.asarray(inputs['W_k'][i]), 4)
        wv = chunk(np.asarray(inputs['W_v'][i]), 4)
        wo = chunk(np.asarray(inputs['W_o'][i]), 4)
        wqkvo = np.concatenate([wq, wk, wv, wo], 1)

        # interleave a/g columns of W_geglu so each 256-col group is (a_mm|g_mm)
        wg = chunk(np.asarray(inputs['W_geglu'][i]), 4)        # [128, 4, 4096]
        wg4 = wg.reshape(128, 4, 2, 16, 128)                   # [., ., a/g, mm, col]
        wg_i = np.ascontiguousarray(
            wg4.transpose(0, 1, 3, 2, 4).reshape(128, 4, 4096))
        bg = np.asarray(inputs['b_geglu'][i], f32).reshape(2, 16, 128)
        bgl = np.ascontiguousarray(
            bg.transpose(2, 1, 0).reshape(128, 32))            # [128, 32] cols (2mm, 2mm+1)
        bgl[:, 0::2] *= ASC                                    # a-bias pre-scaled

        wf_i = chunk(np.asarray(inputs['W_ffout'][i]), 16)
        import ml_dtypes
        wqkvo = wqkvo.astype(ml_dtypes.bfloat16)
        if FP8_AG:
            wg_i = (wg_i * WSC).astype(ml_dtypes.float8_e3m4)
        else:
            wg_i = wg_i.astype(np.float16)
        wf_i = (wf_i / ASC).astype(np.float16)
        blocks.append(dict(
            wqkvo=wqkvo,
            wg=wg_i,
            wf=wf_i,
            bq=np.asarray(inputs['b_q'][i], f32),   # sliced per core below
            bgl=bgl,
            bop=np.ascontiguousarray(bo_p[None]).astype(np.float16),
            bff=np.ascontiguousarray(
                np.asarray(inputs['b_ffout'][i], f32)[None]).astype(np.float16),
            tabb=tabb,
        ))
    return ft, blocks


def kernel(**inputs):
    import os
    n_blocks = int(os.environ.get("KERNEL_NBLOCKS", NB))
    ft, blocks = _host_prep(inputs, n_blocks)

    lb = np.full((NFP, NFP), -30.0, np.float32)
    for fq in range(NF):
        for kf in _kv_frames(fq):
            lb[kf, fq] = 0.0

    in_maps = []
    for c in range(NCORE):
        x0p = np.zeros((TOKP, D), np.float32)
        for f in range(NF):
            x0p[f * OWN:(f + 1) * OWN] = ft[f, OWN * c:OWN * (c + 1)]
            x0p[NOWN + f] = ft[f, S]
        m = {"x0": x0p, "lbias": lb}
        for i in range(n_blocks):
            for k, v in blocks[i].items():
                if k == "bq":
                    v = np.ascontiguousarray(v[64 * c:64 * (c + 1)].reshape(64, 1))
                m[f"{k}{i}"] = v
        in_maps.append(m)

    repeat = int(os.environ.get("KERNEL_REPEAT", 1))
    key = (n_blocks, repeat)
    if key not in _CACHE:
        _CACHE[key] = _build(n_blocks, repeat)
    nc = _CACHE[key]
    trace = bool(os.environ.get("KERNEL_TRACE"))
    res = run_bass_kernel_spmd(nc, in_maps, CORE_IDS, trace=trace)
    global LAST_RESULT
    LAST_RESULT = res

    out = np.zeros((NF, TPF, D), np.float32)
    for c in range(NCORE):
        xo = res.results[c]["xout"]
        for f in range(NF):
            out[f, OWN * c:OWN * (c + 1)] = xo[f * OWN:(f + 1) * OWN]
    x0 = res.results[0]["xout"]
    for f in range(NF):
        out[f, S] = x0[NOWN + f]

    f32 = np.float32
    zr = out[:NZ, :S]
    y = zr @ np.asarray(inputs['W_unpatch'], f32) + np.asarray(inputs['b_unpatch'], f32)
    y = y.reshape(1, NZ, HH // P2, WW // P2, C, P2, P2)
    y = y.transpose(0, 1, 4, 2, 5, 3, 6).reshape(1, NZ, C, HH, WW)
    return np.ascontiguousarray(y.astype(np.float32))



# revision 64
# speedup vs baseline: 1.0087x; 1.0087x over previous
"""Trainium2 Bass kernel for nn_CausalDit (sparse frame-causal DiT).

Sharding over 8 NeuronCores (SPMD, one program):
- Token space: 11 frames (6 noisy "zr" + 5 clean "xa"), 257 tokens each
  (256 patch tokens + 1 register/action token). Core c owns rows
  [32c, 32c+32) of every frame plus a replicated copy of each frame's
  leftover token. Per-core token order: [f*32+j for f, j] (352 own rows,
  frame-major), then 11 leftovers, then 1 pad = 364 rows.
- Dense compute (LN/AdaLN-mod, QKV, Wo, GEGLU FFN, gates) is token-sharded;
  attention is head-sharded (8 heads <-> 8 cores), block-sparse at frame
  granularity, using softmax-without-max (denominator via a ones-column in
  V_aug).
- Split-stream pipelining hides the four AllToAlls per block: tokens split
  into half A (xa frames + all leftover/register slots, cols 192:364) and
  half B (zr own rows, cols 0:192). Order: mod1-A -> QKV-A -> A2A#1;
  mod1-B/QKV-B fill A2A#1's flight; xa self-attention (all from A) fills
  A2A#2's; zr attention covers bk#1 (xa outputs); Wo-A/mod2-A and the
  A-column half of the first two GEGLU pieces cover bk#2 (zr outputs).
  A2A send staging is chunked per d-quarter so collectives trigger right
  after the last QKV matmul; a tiny warmup AllToAll at kernel start absorbs
  the ~25us first-collective ncfw cost.
- Per-frame AdaLN tables are host-precomputed in token-broadcast form
  ([128, 18, D] fp16) and DMA-loaded (no PE one-hot broadcasts). LN rsqrt
  runs as scalar Ln/Exp (no 1.1us DVE reciprocal); b_o'/b_ffout seed their
  PSUM accumulators via ones-row matmuls.
- Attention transport/matmuls bf16, FFN f16. (fp8 e3m4 was tried and
  rejected: no residual skip => activation quantization compounds to
  ~3e-2/block.)
- Last block computes only what reaches the output: zr own rows (xa
  tokens/leftovers are dead past their K/V contribution).
- Host (numpy, fp32) does only tiny prep: patchify + patch matmul,
  embedding gathers, AdaLN tables, bias folds (b_k dropped -
  softmax-shift-invariant; b_v folded into b_o), and the final unpatch.
"""
import numpy as np

import concourse.bass as bass
import concourse.mybir as mybir
import concourse.tile as tile
from concourse import bacc
from concourse.bass_utils import run_bass_kernel_spmd
from concourse.masks import make_identity

# ---- model constants (hardcoded from the problem spec) ----
P2 = 2; NH = 8; NW = 4; NB = 6; D = 512; HID = 2048
HH = 32; WW = 32; C = 3; DUR = 6
DH = D // NH          # 64
NZ = DUR              # 6 zr frames
NX = DUR - 1          # 5 xa frames
NF = NZ + NX          # 11 frames
S = (HH // P2) * (WW // P2)   # 256
TPF = S + 1           # 257 tokens/frame
NCORE = 8
OWN = 32              # owned rows per frame per core
NOWN = NF * OWN       # 352 own rows
TOK = NOWN + NF       # 363 real rows per core
TOKP = TOK + 1        # padded to 364
TPQ = 258             # q columns per frame: 256 own + leftover + pad
NFP = NF + 1          # leftover rows padded to 12
VA = DH + 2           # V_aug cols: 64 v + 1 ones + 1 pad = 66
CORE_IDS = list(range(NCORE))
TOKT = [(0, 128), (128, 128), (256, 108)]   # token tiles of 364
F32 = mybir.dt.float32
F32R = mybir.dt.float32r
BF16 = mybir.dt.bfloat16
AX = mybir.AxisListType.X
ALU = mybir.AluOpType
ACTF = mybir.ActivationFunctionType

F16 = mybir.dt.float16
F8 = mybir.dt.float8e3                   # TRN E3M4: 4 mantissa bits, +-15.5
DTA = BF16                               # attention transport dtype
DTF = F16                                # FFN hp / W_ffout dtype
DT = DTA
DTT = DTA                                # PE-transpose dtype (V path)
# GEGLU a/g matmuls run double-pumped fp8 (e3m4): weights are host-scaled
# by WSC (so sigma~0.02 weights land in e3m4 normals), activations cast
# unscaled, and the 1/WSC folds into the gelu scale / a-bias / W_ffout
import os as _os_mod
# fp8 GEGLU measured rel-err ~3e-2/block: the architecture has no residual
# skip (x is replaced by gate*FFN each block), so activation quantization
# compounds. Keep f16 unless explicitly enabled.
FP8_AG = bool(_os_mod.environ.get("KERNEL_FP8"))
WSC = 16.0 if FP8_AG else 1.0
# xn2 is produced pre-divided by XSC (folded into the mod2 tables) so the
# e3m4 cast cannot saturate (|xn2| can reach ~12 vs e3m4 max 15.5)
XSC = 4.0 if FP8_AG else 1.0
ASC = WSC / XSC
WG_DT = F8 if FP8_AG else F16
XN2_DT = F8 if FP8_AG else F16


def _kv_frames(fq):
    """Global kv-frame indices for q-frame fq (zr: 0..5, xa: 6..10)."""
    if fq < NZ:
        return [fq] + [NZ + j for j in range(max(0, fq - NW), min(fq, NX))]
    return list(range(NZ, fq + 1))


def _R(ap):
    return ap.bitcast(F32R)


def _M(ap):
    """Matmul-operand view: f32 storage is bitcast to f32r; bf16 passes through."""
    return ap.bitcast(F32R) if ap.dtype == F32 else ap


def _ps32(ap):
    """Copy-source view of a PE-transpose PSUM tile (f32r reads as f32)."""
    return ap.bitcast(F32) if ap.dtype == F32R else ap


_CACHE = {}
LAST_RESULT = None


def _build(n_blocks, repeat=1):
    import os as _os
    skip_coll = bool(_os.environ.get("KERNEL_SKIP_COLL"))
    nc = bacc.Bacc("TRN2", target_bir_lowering=False, debug=False,
                   num_devices=NCORE)
    x0_e = nc.declare_dram_parameter("x0", [TOKP, D], F32, isOutput=False)
    lb_e = nc.declare_dram_parameter("lbias", [NFP, NFP], F32, isOutput=False)
    xout_e = nc.declare_dram_parameter("xout", [TOKP, D], F32, isOutput=True)
    ext = []
    for i in range(n_blocks):
        e = dict(
            wqkvo=nc.declare_dram_parameter(f"wqkvo{i}", [128, 16, D], DTA, isOutput=False),
            wg=nc.declare_dram_parameter(f"wg{i}", [128, 4, 2 * HID], WG_DT, isOutput=False),
            wf=nc.declare_dram_parameter(f"wf{i}", [128, 16, D], DTF, isOutput=False),
            bq=nc.declare_dram_parameter(f"bq{i}", [64, 1], F32, isOutput=False),
            bgl=nc.declare_dram_parameter(f"bgl{i}", [128, 32], F32, isOutput=False),
            bop=nc.declare_dram_parameter(f"bop{i}", [1, D], F16, isOutput=False),
            bff=nc.declare_dram_parameter(f"bff{i}", [1, D], F16, isOutput=False),
            # token-broadcast AdaLN tables, host-precomputed: [p, tt*6+v, d]
            tabb=nc.declare_dram_parameter(f"tabb{i}", [128, 18, D], F16, isOutput=False),
        )
        ext.append(e)

    with tile.TileContext(nc) as tc:
        with (
            tc.tile_pool(name="const", bufs=1) as cpool,
            tc.tile_pool(name="blk", bufs=2) as bpool,
            tc.tile_pool(name="xp", bufs=2) as xpool,
            tc.tile_pool(name="wp", bufs=2) as wpool,
            tc.tile_pool(name="wgs", bufs=int(_os.environ.get("KERNEL_WGS", 2))) as wgpool,
            tc.tile_pool(name="act", bufs=1) as apool,
            tc.tile_pool(name="at", bufs=1) as atpool,
            tc.tile_pool(name="hp", bufs=int(_os.environ.get("KERNEL_HP", 2))) as hpool,
            tc.tile_pool(name="sc", bufs=int(_os.environ.get("KERNEL_SC", 2))) as scpool,
            tc.tile_pool(name="pt", bufs=int(_os.environ.get("KERNEL_PT", 6))) as ptpool,
            tc.tile_pool(name="psA", bufs=int(_os.environ.get("KERNEL_PSA", 2)), space="PSUM") as pspool,
            tc.tile_pool(name="psB", bufs=int(_os.environ.get("KERNEL_PSB", 4)), space="PSUM") as big3,
            tc.tile_pool(name="psT", bufs=2, space="PSUM") as pstab,
            tc.tile_pool(name="dram", bufs=1, space="DRAM") as dpool,
        ):
            # warmup collective: absorbs the ~25us first-call ncfw cost and
            # the cross-core entry barrier while the PE does block-0 prep
            warm_sb = cpool.tile([NCORE, 16], DT)
            nc.vector.memset(warm_sb[:NCORE], 0.0)
            warm_s = dpool.tile([NCORE, 16], DT, tag="warms")
            warm_r = dpool.tile([NCORE, 16], DT, tag="warmr")
            nc.sync.dma_start(warm_s[:], warm_sb[:NCORE])
            if not skip_coll:
                nc.gpsimd.collective_compute(
                    "AllToAll", ALU.bypass, replica_groups=[CORE_IDS],
                    ins=[warm_s.opt()], outs=[warm_r.opt()])

            ident32 = cpool.tile([128, 128], F32)
            make_identity(nc, ident32[:])
            ident = cpool.tile([128, 128], F32R)
            nc.vector.tensor_copy(ident[:], ident32[:])
            identT = cpool.tile([128, 128], DTT)
            nc.vector.tensor_copy(identT[:], ident32[:])
            eps = cpool.tile([128, 1], F32)
            nc.vector.memset(eps[:], 1e-5)
            onescol = cpool.tile([1, 128], F16)
            nc.vector.memset(onescol[0:1], 1.0)
            lbias = cpool.tile([NFP, NFP], F32)
            nc.sync.dma_start(lbias[:NFP], lb_e[:])

            for rr in range(repeat):
              x = xpool.tile([128, 3, D], F32, tag="x", name=f"x_{rr}")
              nc.sync.dma_start(x[:, 0:2, :], x0_e[0:256, :].rearrange("(t r) d -> r t d", r=128))
              nc.sync.dma_start(x[:108, 2, :], x0_e[256:364, :])

              for i in range(n_blocks):
                  e = ext[i]
                  wqkvo = wpool.tile([128, 16, D], DTA, tag="wqkvo")
                  nc.sync.dma_start(wqkvo[:], e["wqkvo"][:])
                  bq_sb = bpool.tile([64, 1], F32, tag="bq")
                  nc.sync.dma_start(bq_sb[:64], e["bq"][:])
                  bgl_sb = bpool.tile([128, 32], F32, tag="bgl")
                  nc.sync.dma_start(bgl_sb[:], e["bgl"][:])
                  bop_sb = bpool.tile([1, D], F16, tag="bop")
                  nc.sync.dma_start(bop_sb[0:1], e["bop"][:])
                  bff_sb = bpool.tile([1, D], F16, tag="bff")
                  nc.sync.dma_start(bff_sb[0:1], e["bff"][:])
                  tabb = bpool.tile([128, 18, D], F16, tag="tabb")
                  nc.sync.dma_start(tabb[:], e["tabb"][:])

                  # last block: xa tokens and register/leftover rows are dead
                  # past attention -> keep only zr own rows (cols 0:192)
                  last = i == n_blocks - 1 and rr == repeat - 1
                  tts_live = ([(0, (0, 128)), (1, (128, 64))] if last
                              else list(enumerate(TOKT)))
                  # split-stream row passes: A = xa own + all leftover rows
                  # (token cols 192:364), B = zr own rows (cols 0:192)
                  MOD_A = [(1, 64, 128), (2, 0, 108)]
                  MOD_B = [(0, 0, 128), (1, 0, 64)]
                  WO_A = [] if last else [(1, 64, 128), (2, 0, 96)]
                  WO_B = MOD_B + ([] if last else [(2, 96, 108)])

                  def ln_mod(srcs, v_s, v_t, xn_f32, rows):
                      """xn = LN(srcs[tt])*tab[v_s] + tab[v_t] (fp32).

                      rsqrt via scalar Ln/Exp (DVE reciprocal is ~1.1us);
                      the normalize itself runs on the scalar engine with
                      per-partition scale/bias APs, leaving DVE only the
                      two table ops."""
                      for tt, rl, rh in rows:
                          s_tb = tabb[rl:rh, tt * 6 + v_s, :]
                          t_tb = tabb[rl:rh, tt * 6 + v_t, :]
                          xt = srcs[tt][rl:rh, :]
                          st6 = scpool.tile([128, 6], F32, tag="st6")
                          nc.vector.bn_stats(st6[rl:rh], xt)
                          mv = scpool.tile([128, 2], F32, tag="mv")
                          nc.vector.bn_aggr(mv[rl:rh], st6[rl:rh])
                          lnv = scpool.tile([128, 1], F32, tag="lnv")
                          nc.scalar.activation(lnv[rl:rh], mv[rl:rh, 1:2], ACTF.Ln,
                                               bias=eps[rl:rh])
                          rin = scpool.tile([128, 1], F32, tag="rin")
                          nc.scalar.activation(rin[rl:rh], lnv[rl:rh], ACTF.Exp,
                                               scale=-0.5)
                          # xc overlaps the scalar Ln/Exp pair
                          xc = scpool.tile([128, D], F32, tag="xc")
                          nc.vector.tensor_scalar(xc[rl:rh], xt, mv[rl:rh, 0:1], None,
                                                  op0=ALU.subtract)
                          tmp = scpool.tile([128, D], F32, tag="lntmp")
                          nc.vector.scalar_tensor_tensor(
                              tmp[rl:rh], s_tb, rin[rl:rh], xc[rl:rh],
                              op0=ALU.mult, op1=ALU.mult)
                          nc.vector.tensor_add(xn_f32[rl:rh, tt, :], tmp[rl:rh],
                                               t_tb)

                  def transpose_tok(xn_f32, xnT, rows):
                      """xn [128,3,D] f32 -> xnT [128,4,TOKP] (d-major)."""
                      for tt, rl, rh in rows:
                          r0 = TOKT[tt][0]
                          n = rh - rl
                          pst = big3.tile([128, 512], F32R, tag="ps3")
                          for kd in range(4):
                              nc.tensor.transpose(
                                  pst[:, 128 * kd + rl:128 * kd + rh],
                                  xn_f32[rl:rh, tt, kd * 128:(kd + 1) * 128],
                                  ident[rl:rh, rl:rh])
                          nc.vector.tensor_copy(
                              xnT[:, :, r0 + rl:r0 + rh],
                              pst[:].rearrange("p (k c) -> p k c", k=4)[:, :, rl:rh].bitcast(F32))

                  def qkv_pass(c0, c1, a2a_s_h, qkvt_h):
                      w = c1 - c0
                      for m in range(4):
                          for t in range(3):
                              ps = pspool.tile([128, TOKP], F32, tag="psbig")
                              for k in range(4):
                                  nc.tensor.matmul(ps[:, 0:w],
                                                   wqkvo[:, 4 * t + k, m * 128:(m + 1) * 128],
                                                   xnT[:, k, c0:c1], start=(k == 0), stop=(k == 3))
                              nc.vector.tensor_copy(qkvt_h[:, t, m, :], ps[:, 0:w])
                          nc.sync.dma_start(
                              a2a_s_h[2 * m:2 * m + 2, :, :, :].rearrange("d r t x -> (d r) t x"),
                              qkvt_h[:, :, m, :])

                  def g1x_pass(rows):
                      for tt, rl, rh in rows:
                          nc.vector.tensor_mul(g1x_t[tt][rl:rh],
                                               xn1[rl:rh, tt, :].bitcast(F32),
                                               tabb[rl:rh, tt * 6 + 2, :])

                  XAC = TOKP - 192      # 172 cols: xa own + leftovers + pad
                  ZRC = 192             # zr own cols

                  # ---- mod1 pass A (xa + leftovers) -> QKV-A -> A2A#1 ----
                  xn1 = apool.tile([128, 3, D], F32R, tag="xn1")
                  xnT = apool.tile([128, 4, TOKP], DT, tag="xnT")
                  xsrc = [x[:, 0, :], x[:, 1, :], x[:, 2, :]]
                  ln_mod(xsrc, 0, 1, xn1, MOD_A)
                  transpose_tok(xn1, xnT, MOD_A)
                  a2a_sA = dpool.tile([NCORE, 64, 3, XAC], DT, tag=f"a2asA{rr}_{i}")
                  a2a_rA = dpool.tile([NCORE, 64, 3, XAC], DT, tag=f"a2arA{rr}_{i}")
                  qkvtA = apool.tile([128, 3, 4, XAC], DT, tag="qkvtA")
                  qkv_pass(192, TOKP, a2a_sA, qkvtA)
                  if skip_coll:
                      nc.sync.dma_start(a2a_rA[:], a2a_sA[:])
                  else:
                      nc.gpsimd.collective_compute(
                          "AllToAll", ALU.bypass, replica_groups=[CORE_IDS],
                          ins=[a2a_sA.opt()], outs=[a2a_rA.opt()])

                  # ---- overlap A2A#1: gate1-A, mod1 pass B, QKV-B, A2A#2 ----
                  g1x_t = {tt: apool.tile([128, D], F32, tag=f"g1x_{tt}",
                                          name=f"g1x_{tt}")
                           for tt in (range(2) if last else range(3))}
                  if not last:
                      g1x_pass(MOD_A)
                  ln_mod(xsrc, 0, 1, xn1, MOD_B)
                  transpose_tok(xn1, xnT, MOD_B)
                  a2a_sB = dpool.tile([NCORE, 64, 3, ZRC], DT, tag=f"a2asB{rr}_{i}")
                  a2a_rB = dpool.tile([NCORE, 64, 3, ZRC], DT, tag=f"a2arB{rr}_{i}")
                  qkvtB = apool.tile([128, 3, 4, ZRC], DT, tag="qkvtB")
                  qkv_pass(0, 192, a2a_sB, qkvtB)
                  if skip_coll:
                      nc.sync.dma_start(a2a_rB[:], a2a_sB[:])
                  else:
                      nc.gpsimd.collective_compute(
                          "AllToAll", ALU.bypass, replica_groups=[CORE_IDS],
                          ins=[a2a_sB.opt()], outs=[a2a_rB.opt()])
                  g1x_pass(MOD_B)

                  # ---- receive A then B (V first: PE transposes start early) --
                  qkvsA = atpool.tile([64, 3, NCORE, XAC], DT, tag="qkvsA")
                  for t in (2, 1, 0):
                      nc.sync.dma_start(qkvsA[:, t],
                                        a2a_rA[:, :, t, :].rearrange("s r x -> r s x"))
                  qkvsB = atpool.tile([64, 3, NCORE, ZRC], DT, tag="qkvsB")
                  for t in (2, 1, 0):
                      nc.sync.dma_start(qkvsB[:, t],
                                        a2a_rB[:, :, t, :].rearrange("s r x -> r s x"))
                  qsA, ksA, vsA = qkvsA[:, 0], qkvsA[:, 1], qkvsA[:, 2]
                  qsB, ksB, vsB = qkvsB[:, 0], qkvsB[:, 1], qkvsB[:, 2]

                  # prefetch the first GEGLU weight piece now: emitted later,
                  # its DMA trigger would queue behind collective-dependent
                  # loads in the sync FIFO
                  def load_piece(p):
                      wgp = wgpool.tile([128, 4, 512], WG_DT, tag="wgp")
                      nc.sync.dma_start(wgp[:], e["wg"][:, :, 512 * p:512 * (p + 1)])
                      wfp = wgpool.tile([128, 2, D], DTF, tag="wfp")
                      nc.sync.dma_start(wfp[:], e["wf"][:, 2 * p:2 * (p + 1), :])
                      return wgp, wfp
                  piece = {0: load_piece(0)}

                  # ---- assemble A half: xa frames + leftovers ----
                  vtA = atpool.tile([64, NF, S], DT, tag="vtA")
                  ktA = atpool.tile([64, NF, S], DT, tag="ktA")
                  qtA = atpool.tile([64, NF, TPQ], DT, tag="qtA")
                  v_a = atpool.tile([128, 2 * NF, VA], DT, tag="va")
                  nc.vector.memset(v_a[:, :, DH], 1.0)
                  nc.vector.memset(v_a[:, :, DH + 1], 0.0)
                  nc.vector.tensor_copy(
                      vtA[:, NZ:NF].rearrange("p f (s j) -> p f s j", s=NCORE),
                      vsA[:, :, 0:NX * 32].rearrange("p s (f j) -> p f s j", f=NX))
                  for f in range(NZ, NF):
                      pst = big3.tile([128, 128], DTT, tag="ps3")
                      for t2 in range(2):
                          nc.tensor.transpose(
                              pst[:, 64 * t2:64 * t2 + 64],
                              vtA[:, f, 128 * t2:128 * (t2 + 1)],
                              identT[:64, :64])
                      nc.vector.tensor_copy(
                          v_a[:, 2 * f:2 * f + 2, 0:DH],
                          _ps32(pst[:].rearrange("p (t c) -> p t c", t=2)))
                  v_l = atpool.tile([NFP, VA], DT, tag="vl")
                  nc.vector.memset(v_l[:NFP, DH:DH + 1], 1.0)
                  nc.vector.memset(v_l[:NFP, DH + 1:DH + 2], 0.0)
                  pstl = big3.tile([NFP, 64], DTT, tag="ps3")
                  nc.tensor.transpose(pstl[:NFP, :64], _M(vsA[:, 7, NX * 32:XAC]),
                                      identT[:64, :64])
                  nc.vector.tensor_copy(v_l[:NFP, 0:DH], _ps32(pstl[:NFP, :64]))
                  nc.vector.tensor_copy(
                      ktA[:, NZ:NF].rearrange("p f (s j) -> p f s j", s=NCORE),
                      ksA[:, :, 0:NX * 32].rearrange("p s (f j) -> p f s j", f=NX))
                  ktl = atpool.tile([64, NFP], DT, tag="ktl")
                  nc.vector.tensor_copy(ktl[:64, :], ksA[:, 7, NX * 32:XAC])
                  nc.vector.tensor_scalar(
                      qtA[:, NZ:NF, 0:S].rearrange("p f (s j) -> p f s j", s=NCORE),
                      qsA[:, :, 0:NX * 32].rearrange("p s (f j) -> p f s j", f=NX),
                      bq_sb[:64, 0:1], None, op0=ALU.add)
                  nc.vector.tensor_scalar(qtA[:, :, S], qsA[:, 7, NX * 32:NX * 32 + NF],
                                          bq_sb[:64, 0:1], None, op0=ALU.add)
                  nc.vector.memset(qtA[:, :, S + 1], 0.0)

                  # ---- attention ----
                  otA = atpool.tile([64, NF, TPQ], DT, tag="otA")

                  def attn_frame(fq):
                      kvf = _kv_frames(fq)
                      n_main = 2 * len(kvf)
                      ps_o = pspool.tile([VA, TPQ], F32, tag="psbig")
                      ps_l = big3.tile([NFP, TPQ], F32, tag="ps3")
                      nc.tensor.matmul(ps_l[:NFP], _M(ktl[:64, :]),
                                       _M(qtA[:, fq, :]), start=True, stop=True)
                      pl = ptpool.tile([NFP, TPQ], DTT, tag="pt")
                      nc.scalar.activation(pl[:NFP, :], ps_l[:NFP, :], ACTF.Exp,
                                           scale=0.125, bias=lbias[:NFP, fq:fq + 1])
                      nc.tensor.matmul(ps_o[:], _M(v_l[:NFP, :]), pl[:NFP],
                                       start=True, stop=False)
                      av_i = 0
                      for fi in kvf:
                          for t2 in range(2):
                              ps_s = big3.tile([128, TPQ], F32, tag="ps3")
                              nc.tensor.matmul(
                                  ps_s[:],
                                  ktA[:, fi, 128 * t2:128 * (t2 + 1)],
                                  _M(qtA[:, fq, :]), start=True, stop=True)
                              pt = ptpool.tile([128, TPQ], DTT, tag="pt")
                              nc.scalar.activation(pt[:], ps_s[:], ACTF.Exp, scale=0.125)
                              nc.tensor.matmul(ps_o[:], _M(v_a[:, 2 * fi + t2, :]), pt[:],
                                               start=False, stop=(av_i == n_main - 1))
                              av_i += 1
                      # 1/denominator; broadcast on gpsimd (idle between
                      # collective waits at this point)
                      rc = scpool.tile([1, TPQ], F32, tag="rc")
                      nc.vector.reciprocal(rc[0:1], ps_o[DH:DH + 1, :])
                      bc = scpool.tile([64, TPQ], F32, tag="bc")
                      nc.gpsimd.partition_broadcast(bc[:64], rc[0:1, :])
                      nc.vector.tensor_mul(otA[:, fq, :], ps_o[0:DH, :], bc[:64])

                  # xa self-attention first: runs entirely out of half A,
                  # covering A2A#2's flight
                  if not last:
                      for fq in range(NZ, NF):
                          attn_frame(fq)
                      # ---- bk#1: ship xa outputs while zr attention runs ----
                      bk_sendA = atpool.tile([64, NCORE, NX * 32], DT, tag="bksA")
                      nc.vector.tensor_copy(
                          bk_sendA[:].rearrange("p d (f j) -> p d f j", f=NX),
                          otA[:, NZ:NF, 0:S].rearrange("p f (d j) -> p d f j", d=NCORE))
                      bk_sA = dpool.tile([NCORE, 64, NX * 32], DT, tag=f"bksA{rr}_{i}")
                      bk_rA = dpool.tile([NCORE, 64, NX * 32], DT, tag=f"bkrA{rr}_{i}")
                      nc.sync.dma_start(bk_sA.rearrange("d r x -> r d x"), bk_sendA[:])
                      if skip_coll:
                          nc.sync.dma_start(bk_rA[:], bk_sA[:])
                      else:
                          nc.gpsimd.collective_compute(
                              "AllToAll", ALU.bypass, replica_groups=[CORE_IDS],
                              ins=[bk_sA.opt()], outs=[bk_rA.opt()])
                      # load xa attention outputs NOW: emitted later, this DMA
                      # trigger would queue behind bk#2's staging in the sync
                      # FIFO and stall Wo pass A past bk#2
                      xaT = apool.tile([128, 4, TOKP], DT, tag="qt")
                      nc.sync.dma_start(
                          xaT[:, :, 192:352],
                          bk_rA.rearrange("(c p2) r x -> (p2 r) c x", c=4))

                  # ---- assemble B half: zr frames ----
                  nc.vector.tensor_copy(
                      vtA[:, 0:NZ].rearrange("p f (s j) -> p f s j", s=NCORE),
                      vsB[:, :, 0:NZ * 32].rearrange("p s (f j) -> p f s j", f=NZ))
                  for f in range(NZ):
                      pst = big3.tile([128, 128], DTT, tag="ps3")
                      for t2 in range(2):
                          nc.tensor.transpose(
                              pst[:, 64 * t2:64 * t2 + 64],
                              vtA[:, f, 128 * t2:128 * (t2 + 1)],
                              identT[:64, :64])
                      nc.vector.tensor_copy(
                          v_a[:, 2 * f:2 * f + 2, 0:DH],
                          _ps32(pst[:].rearrange("p (t c) -> p t c", t=2)))
                  nc.vector.tensor_copy(
                      ktA[:, 0:NZ].rearrange("p f (s j) -> p f s j", s=NCORE),
                      ksB[:, :, 0:NZ * 32].rearrange("p s (f j) -> p f s j", f=NZ))
                  nc.vector.tensor_scalar(
                      qtA[:, 0:NZ, 0:S].rearrange("p f (s j) -> p f s j", s=NCORE),
                      qsB[:, :, 0:NZ * 32].rearrange("p s (f j) -> p f s j", f=NZ),
                      bq_sb[:64, 0:1], None, op0=ALU.add)

                  # zr attention (covers bk#1's flight)
                  for fq in range(NZ):
                      attn_frame(fq)

                  # ---- bk#2: zr outputs + all leftover outputs ----
                  BKC = ZRC if last else ZRC + NFP
                  bk_sendB = atpool.tile([64, NCORE, ZRC + NFP], DT, tag="bksB")
                  nc.vector.tensor_copy(
                      bk_sendB[:, :, 0:ZRC].rearrange("p d (f j) -> p d f j", f=NZ),
                      otA[:, 0:NZ, 0:S].rearrange("p f (d j) -> p d f j", d=NCORE))
                  if not last:
                      nc.vector.tensor_copy(
                          bk_sendB[:, :, ZRC:ZRC + NF],
                          otA[:, None, :, S].broadcast_to([64, NCORE, NF]))
                      nc.vector.memset(bk_sendB[:, :, ZRC + NF], 0.0)
                  bk_sB = dpool.tile([NCORE, 64, BKC], DT, tag=f"bksB{rr}_{i}")
                  bk_rB = dpool.tile([NCORE, 64, BKC], DT, tag=f"bkrB{rr}_{i}")
                  nc.sync.dma_start(bk_sB.rearrange("d r x -> r d x"),
                                    bk_sendB[:, :, 0:BKC])
                  if skip_coll:
                      nc.sync.dma_start(bk_rB[:], bk_sB[:])
                  else:
                      nc.gpsimd.collective_compute(
                          "AllToAll", ALU.bypass, replica_groups=[CORE_IDS],
                          ins=[bk_sB.opt()], outs=[bk_rB.opt()])

                  # ---- Wo/mod2 pass A during bk#2's flight, pass B after ----
                  if last:
                      xaT = apool.tile([128, 4, TOKP], DT, tag="qt")
                  nc.sync.dma_start(
                      xaT[:, :, 0:192],
                      bk_rB[:, :, 0:ZRC].rearrange("(c p2) r x -> (p2 r) c x", c=4))
                  if not last:
                      nc.sync.dma_start(
                          xaT[:, :, 352:364],
                          bk_rB[:, :, ZRC:ZRC + NFP].rearrange("(c p2) r x -> (p2 r) c x", c=4))

                  x2g = {tt: apool.tile([128, D], F32, tag=f"x2_{tt}",
                                        name=f"x2_{tt}")
                         for tt in (range(2) if last else range(3))}
                  xn2 = apool.tile([128, 3, D], F32R, tag="xn1")
                  xn2T = apool.tile([128, 4, TOKP], XN2_DT, tag="xn2T")

                  def wo_pass(rows):
                      for tt, rl, rh in rows:
                          r0 = TOKT[tt][0]
                          n = rh - rl
                          # PSUM matmul out base must be 0/32/64; DVE bridges
                          # the offset for the leftover sliver (rows 96:108)
                          ob = rl if rl in (0, 32, 64) else 64
                          pso = pspool.tile([128, D], F32, tag="psbig")
                          # bo' seeds the accumulator via a ones-row matmul
                          nc.tensor.matmul(pso[ob:ob + n], onescol[0:1, 0:n],
                                           bop_sb[0:1, :], start=True, stop=False)
                          for k in range(4):
                              nc.tensor.matmul(pso[ob:ob + n],
                                               _M(xaT[:, k, r0 + rl:r0 + rh]),
                                               wqkvo[:, 12 + k, :], start=False, stop=(k == 3))
                          nc.vector.tensor_add(x2g[tt][rl:rh, :], pso[ob:ob + n],
                                               g1x_t[tt][rl:rh, :])

                  wo_pass(WO_A)
                  ln_mod(x2g, 3, 4, xn2, WO_A)
                  transpose_tok(xn2, xn2T, WO_A)
                  wo_pass(WO_B)
                  ln_mod(x2g, 3, 4, xn2, WO_B)
                  # PE base-partition must be 0/32/64: widen the leftover
                  # sliver's transpose to start at 64 (rows 64:96 rewritten
                  # with identical values)
                  TR_B = MOD_B + ([] if last else [(2, 64, 108)])
                  transpose_tok(xn2, xn2T, TR_B)

                  # ---- GEGLU + FF out, streamed in 8 weight pieces ----
                  psf = {tt: big3.tile([128, D], F32, tag="ps3", name=f"psf{rr}_{i}_{tt}")
                         for tt, _ in tts_live}
                  # b_ffout seeds each accumulator via a ones-row matmul
                  for tt, (r0, p_) in tts_live:
                      nc.tensor.matmul(psf[tt][:p_], onescol[0:1, 0:p_],
                                       bff_sb[0:1, :], start=True, stop=False)
                  tok_rs = ((0, 192),) if last else ((0, TOKP),)
                  # psf row-ranges with matching hp column spans
                  PSF_A = [(1, 64, 128), (2, 0, 108)]
                  PSF_B = [(0, 0, 128), (1, 0, 64)]
                  PSF_FULL = [(tt, 0, p_) for tt, (r0, p_) in
                              sorted(tts_live, key=lambda z: -z[0])]
                  def geglu_stage(mm, j, wgp, wfp, psa, psg, gel, hp, cols, rows):
                      for c0, c1 in cols:
                          for k in range(4):
                              nc.tensor.matmul(psa[:, c0:c1], wgp[:, k, 256 * j:256 * j + 128],
                                               xn2T[:, k, c0:c1], start=(k == 0), stop=(k == 3))
                          for k in range(4):
                              nc.tensor.matmul(psg[:, c0:c1], wgp[:, k, 256 * j + 128:256 * j + 256],
                                               xn2T[:, k, c0:c1], start=(k == 0), stop=(k == 3))
                          nc.scalar.activation(gel[:, c0:c1], psg[:, c0:c1], ACTF.Gelu,
                                               scale=1.0 / ASC,
                                               bias=bgl_sb[:, 2 * mm + 1:2 * mm + 2])
                          nc.vector.scalar_tensor_tensor(hp[:, c0:c1], psa[:, c0:c1],
                                                         bgl_sb[:, 2 * mm:2 * mm + 1], gel[:, c0:c1],
                                                         op0=ALU.add, op1=ALU.mult)
                      for tt, rl, rh in rows:
                          r0 = TOKT[tt][0]
                          assert rl in (0, 32, 64)
                          nc.tensor.matmul(psf[tt][rl:rh],
                                           _M(hp[:, r0 + rl:r0 + rh]),
                                           wfp[:, j, :],
                                           start=False, stop=(mm == 15))

                  for p in range(8):
                      wgp, wfp = piece.pop(p)
                      if p + 1 < 8:
                          piece[p + 1] = load_piece(p + 1)
                      for j in range(2):
                          mm = 2 * p + j
                          # pieces 0-1: A columns run during bk#2's flight
                          # (xn2T-A is ready), B columns after bk#2 lands
                          if p < 2 and not last:
                              stages = [(((192, TOKP),), PSF_A),
                                        (((0, 192),), PSF_B)]
                          else:
                              stages = [(tok_rs, PSF_FULL)]
                          psa = pspool.tile([128, TOKP], F32, tag="psbig")
                          # pstab's banks are idle through the GEGLU body;
                          # using them for psg doubles the pipeline depth
                          psg = pstab.tile([128, TOKP], F32, tag="pstab")
                          gel = scpool.tile([128, TOKP], F32, tag="gel")
                          hp = hpool.tile([128, TOKP], DTF, tag="hp")
                          for cols, rows in stages:
                              geglu_stage(mm, j, wgp, wfp, psa, psg, gel, hp,
                                          cols, rows)

                  x_new = xpool.tile([128, 3, D], F32, tag="x")
                  for tt, (r0, p_) in sorted(tts_live, key=lambda z: -z[0]):
                      nc.vector.tensor_mul(x_new[:p_, tt, :], psf[tt][:p_],
                                           tabb[:p_, tt * 6 + 5, :])
                  x = x_new
                  if _os.environ.get("KERNEL_BLOCK_BARRIER"):
                      # optional scheduling barrier between blocks (collective
                      # ordering is data-enforced; barrier-free validated on HW)
                      tc.strict_bb_all_engine_barrier()

            # only zr own rows (token slots 0:192) survive the last block
            nc.sync.dma_start(xout_e[0:128, :], x[:, 0, :])
            nc.sync.dma_start(xout_e[128:192, :], x[:64, 1, :])
    nc.compile()
    return nc


# ----------------------------------------------------------------------
# host side
# ----------------------------------------------------------------------
def _silu(x):
    return x / (1.0 + np.exp(-x))


def _frame_of():
    """frame index of each per-core token slot (12 = pad/zero row)."""
    fr = np.full(TOKP, NF, np.int64)
    fr[:NOWN] = np.arange(NOWN) // OWN
    fr[NOWN:TOK] = np.arange(NF)
    return fr


def _host_prep(inputs, n_blocks):
    f32 = np.float32
    z = np.asarray(inputs['z'], f32)
    frames = np.asarray(inputs['frames'], f32)
    actions = np.asarray(inputs['actions'])
    ts = np.asarray(inputs['ts'])

    def patch(xx):
        b, dur, c, h, w = xx.shape
        xx = xx.reshape(b, dur, c, h // P2, P2, w // P2, P2)
        xx = xx.transpose(0, 1, 3, 5, 2, 4, 6).reshape(b, dur, (h // P2) * (w // P2), c * P2 * P2)
        return xx @ np.asarray(inputs['W_patch'], f32) + np.asarray(inputs['b_patch'], f32)

    pe = np.asarray(inputs['pe_grid'], f32)
    zt = patch(z)[0] + pe[None]
    xt = patch(frames)[0] + pe[None]
    reg = np.asarray(inputs['registers'], f32)
    aemb = np.asarray(inputs['action_emb'], f32)
    temb = np.asarray(inputs['time_emb'], f32)
    a = aemb[actions[0]]

    ft = np.zeros((NF, TPF, D), f32)
    for f in range(NZ):
        ft[f, :S] = zt[f]
        ft[f, S] = reg[0]
    for f in range(NX):
        ft[NZ + f, :S] = xt[f]
        ft[NZ + f, S] = a[f]

    cond = np.zeros((NF, D), f32)
    for f in range(NZ):
        cond[f] = temb[ts[0, f]]
    for f in range(NX):
        cond[NZ + f] = temb[0]
    sc = _silu(cond)

    blocks = []
    for i in range(n_blocks):
        m1 = sc @ np.asarray(inputs['W_mod1'][i], f32) + np.asarray(inputs['b_mod1'][i], f32)
        s1, t1 = np.split(m1, 2, -1)
        m2 = sc @ np.asarray(inputs['W_mod2'][i], f32) + np.asarray(inputs['b_mod2'][i], f32)
        s2, t2 = np.split(m2, 2, -1)
        g1 = cond @ np.asarray(inputs['W_g1'][i], f32) + np.asarray(inputs['b_g1'][i], f32)
        g2 = cond @ np.asarray(inputs['W_g2'][i], f32) + np.asarray(inputs['b_g2'][i], f32)
        bo_p = (np.asarray(inputs['b_o'][i], f32)
                + np.asarray(inputs['b_v'][i], f32) @ np.asarray(inputs['W_o'][i], f32))
        tabf = np.zeros((NFP + 1, 6, D), f32)
        tabf[:NF] = np.stack([1.0 + s1, t1, g1,
                              (1.0 + s2) / XSC, t2 / XSC, g2], 1)
        # token-broadcast form: row p of group tt gets frame fr(tt*128+p)
        fr = _frame_of()
        frp = np.full(3 * 128, NFP, np.int64)
        frp[:TOKP] = np.minimum(fr, NFP)
        tabb = tabf[frp.reshape(3, 128)]            # [3, 128, 6, D]
        tabb = np.ascontiguousarray(
            tabb.transpose(1, 0, 2, 3).reshape(128, 18, D)).astype(np.float16)

        def chunk(w, kparts):
            K, N = w.shape
            return np.ascontiguousarray(
                np.asarray(w, f32).reshape(kparts, 128, N).swapaxes(0, 1))

        wq = chunk(np.asarray(inputs['W_q'][i]), 4)
        wk = chunk(np.asarray(inputs['W_k'][i]), 4)
        wv = chunk(np.asarray(inputs['W_v'][i]), 4)
        wo = chunk(np.asarray(inputs['W_o'][i]), 4)
        wqkvo = np.concatenate([wq, wk, wv, wo], 1)

        # interleave a/g columns of W_geglu so each 256-col group is (a_mm|g_mm)
        wg = chunk(np.asarray(inputs['W_geglu'][i]), 4)        # [128, 4, 4096]
        wg4 = wg.reshape(128, 4, 2, 16, 128)                   # [., ., a/g, mm, col]
        wg_i = np.ascontiguousarray(
            wg4.transpose(0, 1, 3, 2, 4).reshape(128, 4, 4096))
        bg = np.asarray(inputs['b_geglu'][i], f32).reshape(2, 16, 128)
        bgl = np.ascontiguousarray(
            bg.transpose(2, 1, 0).reshape(128, 32))            # [128, 32] cols (2mm, 2mm+1)
        bgl[:, 0::2] *= ASC                                    # a-bias pre-scaled

        wf_i = chunk(np.asarray(inputs['W_ffout'][i]), 16)
        import ml_dtypes
        wqkvo = wqkvo.astype(ml_dtypes.bfloat16)
        if FP8_AG:
            wg_i = (wg_i * WSC).astype(ml_dtypes.float8_e3m4)
        else:
            wg_i = wg_i.astype(np.float16)
        wf_i = (wf_i / ASC).astype(np.float16)
        blocks.append(dict(
            wqkvo=wqkvo,
            wg=wg_i,
            wf=wf_i,
            bq=np.asarray(inputs['b_q'][i], f32),   # sliced per core below
            bgl=bgl,
            bop=np.ascontiguousarray(bo_p[None]).astype(np.float16),
            bff=np.ascontiguousarray(
                np.asarray(inputs['b_ffout'][i], f32)[None]).astype(np.float16),
            tabb=tabb,
        ))
    return ft, blocks


def kernel(**inputs):
    import os
    n_blocks = int(os.environ.get("KERNEL_NBLOCKS", NB))
    ft, blocks = _host_prep(inputs, n_blocks)

    lb = np.full((NFP, NFP), -30.0, np.float32)
    for fq in range(NF):
        for kf in _kv_frames(fq):
            lb[kf, fq] = 0.0

    in_maps = []
    for c in range(NCORE):
        x0p = np.zeros((TOKP, D), np.float32)
        for f in range(NF):
            x0p[f * OWN:(f + 1) * OWN] = ft[f, OWN * c:OWN * (c + 1)]
            x0p[NOWN + f] = ft[f, S]
        m = {"x0": x0p, "lbias": lb}
        for i in range(n_blocks):
            for k, v in blocks[i].items():
                if k == "bq":
                    v = np.ascontiguousarray(v[64 * c:64 * (c + 1)].reshape(64, 1))
                m[f"{k}{i}"] = v
        in_maps.append(m)

    repeat = int(os.environ.get("KERNEL_REPEAT", 1))
    key = (n_blocks, repeat)
    if key not in _CACHE:
        _CACHE[key] = _build(n_blocks, repeat)
    nc = _CACHE[key]
    trace = bool(os.environ.get("KERNEL_TRACE"))
    res = run_bass_kernel_spmd(nc, in_maps, CORE_IDS, trace=trace)
    global LAST_RESULT
    LAST_RESULT = res

    out = np.zeros((NF, TPF, D), np.float32)
    for c in range(NCORE):
        xo = res.results[c]["xout"]
        for f in range(NF):
            out[f, OWN * c:OWN * (c + 1)] = xo[f * OWN:(f + 1) * OWN]
    x0 = res.results[0]["xout"]
    for f in range(NF):
        out[f, S] = x0[NOWN + f]

    f32 = np.float32
    zr = out[:NZ, :S]
    y = zr @ np.asarray(inputs['W_unpatch'], f32) + np.asarray(inputs['b_unpatch'], f32)
    y = y.reshape(1, NZ, HH // P2, WW // P2, C, P2, P2)
    y = y.transpose(0, 1, 4, 2, 5, 3, 6).reshape(1, NZ, C, HH, WW)
    return np.ascontiguousarray(y.astype(np.float32))

